# revision 1
# baseline (speedup 1.0000x reference)
"""GPT-2 style attention block (B=8, S=1024, NX=1024, H=16, D=64) on 8 TRN2
NeuronCores, data-parallel over batch (one batch element per core).

Per-core math (batch element b):
  qkv = x @ w_attn + b_attn ; split q,k,v ; per head: softmax(causal(q k^T / 8)) v
  out = merge_heads @ w_proj + b_proj

Layout strategy (single core, no collectives):
  - xT built via PE transposes so the contraction dim (NX) sits on partitions.
  - q,k computed TRANSPOSED (qkT[n, s]) so each head's qT/kT slices have the
    head dim (64) on partitions: exactly the operands scoresT = k q^T needs.
  - scores computed transposed: ST[sk, sq]; the exp and the PV matmuls only
    touch the causal (sq >= sk) column ranges, so fully/partially masked
    regions are never computed or zero-filled.
  - v computed in natural layout [sk, (h, d)] with an all-ones column per
    head: PV matmul U^T = [v | 1]^T E^T yields the softmax denominator as
    its last row for free.
  - normalize: DVE reciprocal of that row, PE outer-product broadcast, DVE
    multiply straight into aT (which is already the lhsT layout for w_proj).

All matmuls run in bf16 (inputs rounded to bf16, fp32 PSUM accumulation):
measured end-to-end rel err ~3e-3 vs the fp32 reference.
Pools are phase-scoped so peak SBUF stays under the allocator cap.
"""

import numpy as np

B, S, NX, H = 8, 1024, 1024, 16
D = NX // H          # 64
P = 128              # partitions
ST = S // P          # 8 s-tiles
KT = NX // P         # 8 k-tiles
NQK = 2 * NX // P    # 16 n-tiles covering q and k
CH = 512             # matmul free-dim chunk (one PSUM bank of fp32)
NCH = S // CH        # 2 chunks
E = D + 1            # v columns per head incl. ones column


def _split_excess_waits(nc):
    """Post-scheduling pass: the TPB instruction encodings carry at most one
    embedded sync-wait (and matmuls with their fused weight-load carry none),
    but Tile may attach several.  Move excess waits onto InstNoOp instructions
    inserted immediately before, on the same engine — semantically identical,
    but walrus can encode it."""
    import concourse.mybir as mybir

    SKIP = {
        "InstEventSemaphore",
        "InstUnconditionalBranch",
        "InstConditionalBranch",
        "InstRegisterMove",
        "InstRegisterAluOp",
    }
    n = 0
    for fn in nc.m.functions:
        for bb in fn.blocks:
            insts = bb.instructions
            inserts = []  # (index, [nops])
            for i, inst in enumerate(insts):
                tname = type(inst).__name__
                if tname in SKIP:
                    continue
                si = inst.sync_info
                if si is None or not si.on_wait:
                    continue
                waits = list(si.on_wait)
                cap = 1
                if len(waits) <= cap:
                    continue
                keep, move = waits[:cap], waits[cap:]
                nops = []
                for w in move:
                    n += 1
                    nops.append(
                        mybir.InstNoOp(
                            name=f"wsplit-{n}",
                            text_hint="wsplit",
                            bass_nofuse=True,
                            engine=inst.engine,
                            sync_info=mybir.SyncInfo(on_wait=[w], on_update=[]),
                        )
                    )
                inst.sync_info = mybir.SyncInfo(
                    on_wait=keep,
                    on_update=list(si.on_update) if si.on_update else [],
                )
                inserts.append((i, nops))
            for i, nops in reversed(inserts):
                for nop in reversed(nops):
                    insts.insert(i, nop)
                    try:
                        nc.register_instruction(nop, overwrite=True)
                    except Exception:
                        pass
    return n


def build_nc():
    import concourse.bass as bass
    import concourse.mybir as mybir
    from concourse.tile import TileContext
    from concourse.masks import make_identity, make_upper_triangular

    f32 = mybir.dt.float32
    bf16 = mybir.dt.bfloat16
    Exp = mybir.ActivationFunctionType.Exp

    nc = bass.Bass(target_bir_lowering=False)
    x_ext = nc.declare_dram_parameter("x", [S, NX], f32, isOutput=False)
    wa_ext = nc.declare_dram_parameter("w_attn", [NX, 3 * NX], f32, isOutput=False)
    ba_ext = nc.declare_dram_parameter("b_attn", [3 * NX], f32, isOutput=False)
    wp_ext = nc.declare_dram_parameter("w_proj", [NX, NX], f32, isOutput=False)
    bp_ext = nc.declare_dram_parameter("b_proj", [NX], f32, isOutput=False)
    out_ext = nc.declare_dram_parameter("out", [S, NX], f32, isOutput=True)

    with TileContext(nc) as tc:
        with (
            tc.tile_pool(name="const", bufs=1) as const,
            tc.tile_pool(name="small", bufs=3) as small,
        ):
            # Manually-managed phase pools (must close in LIFO order):
            cm_scr2 = tc.tile_pool(name="scr2", bufs=1)      # xT then aT
            scr2 = cm_scr2.__enter__()
            cm_wp = tc.tile_pool(name="pool_wp", bufs=1)     # w_proj (bf16)
            pool_wp = cm_wp.__enter__()
            cm_qkv = tc.tile_pool(name="pool_qkv", bufs=1)   # qkT + v
            pool_qkv = cm_qkv.__enter__()
            cm_psAB = tc.tile_pool(name="ps_ab", bufs=1, space="PSUM")
            ps = cm_psAB.__enter__()                         # phases A..B2
            cm_scr1 = tc.tile_pool(name="scr1", bufs=1)      # x (phase A)
            scr1 = cm_scr1.__enter__()
            # ---------------- constants ----------------
            ident = const.tile([P, P], f32)
            make_identity(nc, ident)
            mask01 = const.tile([P, P], bf16)   # keep sq >= sk (incl diag)
            make_upper_triangular(nc, mask01, val=1.0, diag=True)
            ones_row = const.tile([1, P], bf16)
            nc.vector.memset(ones_row, 1.0)
            ba_v = const.tile([1, NX], bf16)    # b_attn[2048:3072] (v bias)
            nc.gpsimd.dma_start(out=ba_v, in_=ba_ext[2 * NX : 3 * NX].unsqueeze(0))
            ba_col = const.tile([P, NQK], f32)  # b_attn[:2048] column-major
            nc.sync.dma_start(
                out=ba_col, in_=ba_ext[0 : 2 * NX].rearrange("(nt p) -> p nt", p=P)
            )
            bp_row = const.tile([1, NX], bf16)
            nc.gpsimd.dma_start(out=bp_row, in_=bp_ext[:].unsqueeze(0))

            scratch1 = scr1.tile([P, ST * NX], f32)    # x (phase A only)
            scratch2 = scr2.tile([P, KT * S], bf16)    # xT, later aT

            # ---------------- phase A: x, xT ----------------
            x_sb = scratch1
            for st in range(ST):
                nc.sync.dma_start(
                    out=x_sb[:, st * NX : (st + 1) * NX],
                    in_=x_ext[st * P : (st + 1) * P, :],
                )
            xT = scratch2
            for st in range(ST):
                pt = ps.tile([P, KT * P], f32, name="pm2", bufs=4)
                for kt in range(KT):
                    nc.tensor.transpose(
                        out=pt[:, kt * P : (kt + 1) * P],
                        in_=x_sb[:, st * NX + kt * P : st * NX + (kt + 1) * P],
                        identity=ident,
                    )
                nc.scalar.copy(
                    out=bass.AP(
                        tensor=xT.tensor,
                        offset=xT.offset + st * P,
                        ap=[[KT * S, P], [S, KT], [1, P]],
                    ),
                    in_=pt.rearrange("p (kt n) -> p kt n", n=P),
                )
            cm_scr1.__exit__(None, None, None)

            # ---------------- phase B: qkT = (x @ w_qk)^T + b ----------------
            qkT = pool_qkv.tile([P, NQK * S], bf16)
            with tc.tile_pool(name="pool_wsl", bufs=3) as pool_wsl:
                for nt in range(NQK):
                    wstage = pool_wsl.tile([P, KT * P], f32, name="wstage")
                    nc.sync.dma_start(
                        out=wstage.rearrange("p (kt n) -> p kt n", n=P),
                        in_=wa_ext.rearrange("(kt p) n -> p kt n", p=P)[
                            :, :, nt * P : (nt + 1) * P
                        ],
                    )
                    wsl = pool_wsl.tile([P, KT * P], bf16, name="wsl")
                    nc.scalar.copy(out=wsl, in_=wstage)
                    pm = ps.tile([P, S], f32, name="pm2", bufs=4)
                    for kt in range(KT):
                        for c in range(NCH):
                            nc.tensor.matmul(
                                out=pm[:, c * CH : (c + 1) * CH],
                                lhsT=wsl[:, kt * P : (kt + 1) * P],
                                rhs=xT[:, kt * S + c * CH : kt * S + (c + 1) * CH],
                                start=(kt == 0),
                                stop=(kt == KT - 1),
                            )
                    nc.scalar.add(
                        out=qkT[:, nt * S : (nt + 1) * S],
                        in_=pm,
                        add=ba_col[:, nt : nt + 1],
                    )

            # ---------------- phase B2: v natural [sk, (h, d|1)] ----------------
            cm_wv = tc.tile_pool(name="pool_wv", bufs=1)
            pool_wv = cm_wv.__enter__()
            wv = pool_wv.tile([P, KT * NX], bf16)
            with tc.tile_pool(name="pool_wvs", bufs=2) as pool_wvs:
                for kt in range(KT):
                    wvs = pool_wvs.tile([P, NX], f32, name="wvs")
                    nc.sync.dma_start(
                        out=wvs,
                        in_=wa_ext.rearrange("(kt p) n -> p kt n", p=P)[
                            :, kt, 2 * NX : 3 * NX
                        ],
                    )
                    nc.scalar.copy(out=wv[:, kt * NX : (kt + 1) * NX], in_=wvs)
            v_sb = pool_qkv.tile([P, ST * H * E], bf16)
            nc.vector.memset(
                v_sb.rearrange("p (st h e) -> p st h e", h=H, e=E)[:, :, :, D : D + 1],
                1.0,
            )
            for st in range(ST):
                pm = ps.tile([P, NX], f32, name="pm2", bufs=4)
                for kt in range(KT):
                    for c in range(NCH):
                        nc.tensor.matmul(
                            out=pm[:, c * CH : (c + 1) * CH],
                            lhsT=xT[:, kt * S + st * P : kt * S + (st + 1) * P],
                            rhs=wv[:, kt * NX + c * CH : kt * NX + (c + 1) * CH],
                            start=(kt == 0),
                            stop=False,
                        )
                for c in range(NCH):
                    nc.tensor.matmul(  # + b_attn[2048 + c*CH : ...] over all rows
                        out=pm[:, c * CH : (c + 1) * CH],
                        lhsT=ones_row,
                        rhs=ba_v[:, c * CH : (c + 1) * CH],
                        start=False,
                        stop=True,
                    )
                nc.vector.tensor_copy(
                    out=v_sb.rearrange("p (st h e) -> p st h e", h=H, e=E)[
                        :, st, :, 0:D
                    ],
                    in_=pm.rearrange("p (h d) -> p h d", d=D),
                )
            cm_wv.__exit__(None, None, None)
            cm_psAB.__exit__(None, None, None)

            # ---------------- phase C: attention, head pairs ----------------
            # Heads 2t (partitions 0-63 of qkT) and 2t+1 (partitions 64-127)
            # are processed together: their scores matmuls alternate in the
            # stream and run CONCURRENTLY on disjoint PE row-groups.  The PV
            # matmuls of pair t-1 are interleaved between the scores of pair
            # t so the PE never waits for the exp drain, and the causal mask
            # is applied in two half-range ops per head to keep the PV
            # dependency chain fine-grained.
            aT = scratch2   # overwrites xT (WAR dep handled by Tile)
            with (
                tc.tile_pool(name="pool_et", bufs=4) as pool_et,
                tc.tile_pool(name="ps_c", bufs=1, space="PSUM") as ps,
            ):
                state = {}

                def pv_mm_list(h):
                    mms = []
                    for kt in range(KT):
                        for c in range(NCH):
                            kt_hi = min(KT, ((c + 1) * CH) // P)
                            if kt >= kt_hi:
                                continue
                            off = max(0, P * kt - c * CH)
                            mms.append((c, kt, off, kt == 0, kt == kt_hi - 1))
                    return mms

                def emit_pv_mms(h, ET, pu, mms):
                    for c, kt, off, first, last in mms:
                        nc.tensor.matmul(
                            out=pu[:, c * CH + off : (c + 1) * CH],
                            lhsT=v_sb[:, (kt * H + h) * E : (kt * H + h) * E + E],
                            rhs=ET[
                                :, kt * S + c * CH + off : kt * S + (c + 1) * CH
                            ],
                            start=first,
                            stop=last,
                        )

                def emit_mask_half(ET, lo, hi):
                    # diagonal blocks kt in [lo, hi): one strided DVE multiply
                    diag = bass.AP(
                        tensor=ET.tensor,
                        offset=ET.offset + lo * (S + P),
                        ap=[[KT * S, P], [S + P, hi - lo], [1, P]],
                    )
                    nc.vector.tensor_mul(
                        out=diag,
                        in0=diag,
                        in1=mask01.unsqueeze(1).broadcast_to((P, hi - lo, P)),
                    )

                def emit_pair(t):
                    heads = (2 * t, 2 * t + 1)
                    ETs = {}
                    for h in heads:
                        ETs[h] = pool_et.tile([P, KT * S], bf16, name="ET")
                    prev = state.pop(t - 1, None)
                    if prev is not None:
                        prev_ETs, prev_pus, prev_mms = prev
                        for h in prev_ETs:
                            prev_pus[h] = ps.tile([E, S], f32, name="pu", bufs=2)
                    for kt in range(KT):
                        dstart = P * kt
                        for c in range(dstart // CH, NCH):
                            off = max(0, dstart - c * CH)
                            pms = {}
                            for h in heads:
                                pms[h] = ps.tile([P, CH], f32, name="pm1", bufs=4)
                                po = (h % 2) * 64
                                qbase = (h // 2) * S
                                kbase = (H // 2 + h // 2) * S
                                nc.tensor.matmul(
                                    out=pms[h][:, off:CH],
                                    lhsT=qkT[
                                        po : po + 64,
                                        kbase + kt * P : kbase + (kt + 1) * P,
                                    ],
                                    rhs=qkT[
                                        po : po + 64,
                                        qbase + c * CH + off : qbase + (c + 1) * CH,
                                    ],
                                    start=True,
                                    stop=True,
                                )
                            for h in heads:
                                nc.scalar.activation(
                                    out=ETs[h][
                                        :,
                                        kt * S + c * CH + off : kt * S + (c + 1) * CH,
                                    ],
                                    in_=pms[h][:, off:CH],
                                    func=Exp,
                                    scale=0.125,
                                )
                        if kt == 3 or kt == 7:
                            for h in heads:
                                emit_mask_half(ETs[h], kt - 3, kt + 1)
                        # interleave PV matmuls of the previous pair
                        if prev is not None:
                            take = 3
                            chunk, prev_mms = prev_mms[:take], prev_mms[take:]
                            for hh, c, kt2, off2, first, last in chunk:
                                emit_pv_mms(
                                    hh,
                                    prev_ETs[hh],
                                    prev_pus[hh],
                                    [(c, kt2, off2, first, last)],
                                )
                    if prev is not None:
                        for hh, c, kt2, off2, first, last in prev_mms:
                            emit_pv_mms(
                                hh,
                                prev_ETs[hh],
                                prev_pus[hh],
                                [(c, kt2, off2, first, last)],
                            )
                        for h in prev_ETs:
                            finish_pv(h, prev_pus[h])
                    mms = []
                    for h in heads:
                        mms.extend((h,) + m for m in pv_mm_list(h))
                    # alternate the two heads' PV matmuls for row... (K=128,
                    # no row concurrency, but keeps both ET tiles hot)
                    a = [m for m in mms if m[0] == heads[0]]
                    b = [m for m in mms if m[0] == heads[1]]
                    inter = []
                    for x, y in zip(a, b):
                        inter.extend((x, y))
                    state[t] = (ETs, {}, inter)

                def finish_pv(h, pu):
                    """reciprocal + broadcast + normalized write into aT.

                    The denominators sit in one PSUM row; a [1, S] reciprocal
                    would run on a single DVE lane (~6.5us).  Repartition to
                    [128, S/128] with two tiny DMAs so all lanes work."""
                    po = (h % 2) * 64
                    r_sb = small.tile([1, S], f32, name="r_sb")
                    nc.vector.tensor_copy(out=r_sb, in_=pu[D : D + 1, :])
                    r_wide = small.tile([P, S // P], f32, name="r_wide")
                    nc.sync.dma_start(out=r_wide, in_=r_sb)
                    rec_wide = small.tile([P, S // P], bf16, name="rec_wide")
                    with nc.allow_low_precision(
                        reason="softmax denominators; bf16 ok at 2e-2 gate"
                    ):
                        nc.vector.reciprocal(out=rec_wide, in_=r_wide)
                    recip = small.tile([1, S], bf16, name="recip")
                    nc.sync.dma_start(out=recip, in_=rec_wide)
                    for c in range(NCH):
                        pr = ps.tile([64, CH], f32, name="pm1", bufs=4)
                        nc.tensor.matmul(
                            out=pr,
                            lhsT=ones_row[:, 0:64],
                            rhs=recip[:, c * CH : (c + 1) * CH],
                            start=True,
                            stop=True,
                        )
                        recipB = small.tile([64, CH], f32, name="recipB")
                        nc.vector.tensor_copy(out=recipB, in_=pr)
                        nc.vector.tensor_mul(
                            out=aT[
                                po : po + 64,
                                (h // 2) * S + c * CH : (h // 2) * S + (c + 1) * CH,
                            ],
                            in0=pu[0:D, c * CH : (c + 1) * CH],
                            in1=recipB,
                        )

                wp_sb = pool_wp.tile([P, KT * NX], bf16)
                for t in range(H // 2):
                    emit_pair(t)
                    if t == H // 2 - 2:
                        # prefetch w_proj while the last pair's exp drains
                        with tc.tile_pool(name="pool_wps", bufs=2) as pool_wps:
                            for kt in range(KT):
                                wps = pool_wps.tile([P, NX], f32, name="wps")
                                nc.sync.dma_start(
                                    out=wps,
                                    in_=wp_ext.rearrange("(kt p) n -> p kt n", p=P)[
                                        :, kt, :
                                    ],
                                )
                                nc.scalar.copy(
                                    out=wp_sb[:, kt * NX : (kt + 1) * NX], in_=wps
                                )
                # drain the last pair
                last_ETs, last_pus, last_mms = state.pop(H // 2 - 1)
                for h in last_ETs:
                    last_pus[h] = ps.tile([E, S], f32, name="pu", bufs=2)
                for hh, c, kt2, off2, first, last in last_mms:
                    emit_pv_mms(hh, last_ETs[hh], last_pus[hh], [(c, kt2, off2, first, last)])
                for h in last_ETs:
                    finish_pv(h, last_pus[h])

            cm_qkv.__exit__(None, None, None)

            # ---------------- phase D: out = a @ w_proj + b_proj ----------------
            cm_psD = tc.tile_pool(name="ps_d", bufs=1, space="PSUM")
            ps = cm_psD.__enter__()
            for st in range(ST):
                pm = ps.tile([P, NX], f32, name="pm2", bufs=4)
                for kt in range(KT):
                    for c in range(NCH):
                        nc.tensor.matmul(
                            out=pm[:, c * CH : (c + 1) * CH],
                            lhsT=aT[:, kt * S + st * P : kt * S + (st + 1) * P],
                            rhs=wp_sb[:, kt * NX + c * CH : kt * NX + (c + 1) * CH],
                            start=(kt == 0),
                            stop=False,
                        )
                for c in range(NCH):
                    nc.tensor.matmul(
                        out=pm[:, c * CH : (c + 1) * CH],
                        lhsT=ones_row,
                        rhs=bp_row[:, c * CH : (c + 1) * CH],
                        start=False,
                        stop=True,
                    )
                stage = small.tile([P, NX], f32, name="stage", bufs=3)
                nc.vector.tensor_copy(out=stage, in_=pm)
                nc.sync.dma_start(
                    out=out_ext[st * P : (st + 1) * P, :],
                    in_=stage,
                )
            cm_psD.__exit__(None, None, None)
            cm_wp.__exit__(None, None, None)
            cm_scr2.__exit__(None, None, None)

    _split_excess_waits(nc)
    return nc


def _enable_ldw_opt():
    """walrus is invoked with --enable-ldw-opt=false on this path; turning it
    on lets codegen elide redundant LDWEIGHTS for back-to-back matmuls that
    share a stationary operand."""
    import concourse.bass_utils as bu

    if getattr(bu, "_ldw_opt_patched", False):
        return
    orig = bu.run_command

    def patched(cmd, **kw):
        cmd = [
            c.replace("--enable-ldw-opt=false", "--enable-ldw-opt=true")
            if isinstance(c, str)
            else c
            for c in cmd
        ]
        return orig(cmd, **kw)

    bu.run_command = patched
    bu._ldw_opt_patched = True


def run(inputs, trace=False, **kwargs):
    """Run the SPMD kernel on 8 cores; returns (output, BassKernelResults)."""
    from concourse.bass_utils import run_bass_kernel_spmd


    x = np.ascontiguousarray(np.asarray(inputs["x"], dtype=np.float32))
    w_attn = np.ascontiguousarray(np.asarray(inputs["w_attn"], dtype=np.float32))
    b_attn = np.ascontiguousarray(np.asarray(inputs["b_attn"], dtype=np.float32))
    w_proj = np.ascontiguousarray(np.asarray(inputs["w_proj"], dtype=np.float32))
    b_proj = np.ascontiguousarray(np.asarray(inputs["b_proj"], dtype=np.float32))

    nc = build_nc()
    in_maps = [
        {
            "x": x[b],
            "w_attn": w_attn,
            "b_attn": b_attn,
            "w_proj": w_proj,
            "b_proj": b_proj,
        }
        for b in range(B)
    ]
    res = run_bass_kernel_spmd(
        nc, in_maps, core_ids=list(range(B)), trace=trace, **kwargs
    )
    out = np.stack([res.results[i]["out"] for i in range(B)], axis=0)
    return out.astype(np.float32), res


def kernel(**inputs):
    out, _ = run(inputs)
    return out

